# revision 10
# baseline (speedup 1.0000x reference)
"""Trainium2 Bass kernel for CAM (channel attention module).

Per batch b: A = x[b] reshaped [N=4096, C=256]
  aTa  = A^T @ A                      [C, C]
  attn = softmax(aTa, axis=-1)
  out  = gamma * (A @ attn) + x[b]

Identity-free formulation used on device:
  out = (A @ (gamma * attn)) + x     (residual added in fp32 on DVE)

Sharding: data-parallel over batch. 32 batches -> 8 cores x 4 batches.
Matmuls run in bf16 (PE streams 1 col/cycle; fp32 would be 4x slower);
the residual add is fp32 so with gamma == 0 the output equals x exactly.

A^T tiles (needed as stationary operand of the second matmul) are produced
on the PE itself via matmul-by-identity (out = lhsT.T @ I), landing in PSUM
and copied/cast to SBUF by the vector engine.
"""

import numpy as np

try:
    import concourse.bass as bass
except ImportError:  # pragma: no cover
    import sys

    sys.path.insert(0, "/opt/trn_rl_repo")
    import concourse.bass as bass

import concourse.bacc as bacc
import concourse.mybir as mybir
import concourse.tile as tile
from concourse.masks import make_identity

# Full problem: x [32, 64, 64, 256] fp32, gamma [1] fp32.
B_FULL = 32
H = W = 64
N = H * W  # 4096
C = 256
P = 128
N_CORES = 8
B_PER_CORE = B_FULL // N_CORES  # 4
KT = N // P  # 32 row-tiles of A per batch
F32 = mybir.dt.float32
BF16 = mybir.dt.bfloat16

def build_bass(reps: int = 1):
    """Build the per-core Bass program (4 batches per core).

    reps > 1 repeats the whole body (same inputs/outputs) — used only for
    delta-timing on hardware; the result is unchanged.
    """
    nc = bacc.Bacc(
        "TRN2",
        target_bir_lowering=False,
        debug=False,
        enable_asserts=True,
        num_devices=1,
    )
    x = nc.dram_tensor("x", [B_PER_CORE, N, C], F32, kind="ExternalInput")
    gamma = nc.dram_tensor("gamma", [1], F32, kind="ExternalInput")
    out = nc.dram_tensor("out", [B_PER_CORE, N, C], F32, kind="ExternalOutput")

    with tile.TileContext(nc) as tc:
        with (
            tc.tile_pool(name="singles", bufs=1) as singles,
            tc.tile_pool(name="xf_pool", bufs=2) as xf_pool,
            tc.tile_pool(name="xb_pool", bufs=2) as xb_pool,
            tc.tile_pool(name="xt_pool", bufs=2) as xt_pool,
            tc.tile_pool(name="sm_pool", bufs=4) as sm_pool,
            tc.tile_pool(name="ex_pool", bufs=4) as ex_pool,
            tc.tile_pool(name="outst_pool", bufs=3) as outst_pool,
            tc.tile_pool(name="ata_pool", bufs=2, space="PSUM") as ata_pool,
            tc.tile_pool(name="tp_pool", bufs=2, space="PSUM") as tp_pool,
            tc.tile_pool(name="op_pool", bufs=2, space="PSUM") as op_pool,
        ):
            ident = singles.tile([P, P], BF16)
            make_identity(nc, ident)
            gamma_sb = singles.tile([P, 1], F32)
            nc.gpsimd.dma_start(out=gamma_sb, in_=gamma[:].to_broadcast((P, 1)))

            for rep in range(reps):
              for b in range(B_PER_CORE):
                u = f"{rep}_{b}"
                # ---- load + cast (4 x 1MiB chunks) -------------------------------
                xf = xf_pool.tile([P, KT, C], F32, tag="xf")
                xv = x[b].rearrange("(k p) c -> p k c", p=P)
                for q in range(4):
                    nc.sync.dma_start(
                        out=xf[:, 8 * q : 8 * (q + 1), :],
                        in_=xv[:, 8 * q : 8 * (q + 1), :],
                    )
                xb = xb_pool.tile([P, KT, C], BF16, tag="xb")
                for j in range(8):
                    nc.scalar.copy(
                        out=xb[:, 4 * j : 4 * j + 4, :],
                        in_=xf[:, 4 * j : 4 * j + 4, :],
                    )

                # ---- phase 1: aTa syrk + A^T transposes --------------------------
                ata = [
                    ata_pool.tile([P, C], F32, tag=f"ata{ch}", name=f"ata{ch}_{u}")
                    for ch in range(2)
                ]
                xt = [
                    xt_pool.tile([P, N], BF16, tag=f"xt{ch}", name=f"xt{ch}_{u}")
                    for ch in range(2)
                ]
                for j in range(4):
                    for kk in range(8):
                        k = 8 * j + kk
                        for ch in range(2):
                            nc.tensor.matmul(
                                ata[ch],
                                lhsT=xb[:, k, ch * P : (ch + 1) * P],
                                rhs=xb[:, k, :],
                                start=(k == 0),
                                stop=(k == KT - 1),
                            )
                    for ch in range(2):
                        tp = tp_pool.tile(
                            [P, 8, P], BF16, tag="tp", name=f"tp{ch}_{u}_{j}"
                        )
                        for kk in range(8):
                            k = 8 * j + kk
                            nc.tensor.transpose(
                                tp[:, kk, :],
                                xb[:, k, ch * P : (ch + 1) * P],
                                ident,
                            )
                        # PSUM -> SBUF evacuation: ch0 on ACT, ch1 on DVE
                        # (DVE gets the 2x bf16 mode; ACT offloads the rest)
                        if ch == 0:
                            nc.scalar.copy(
                                out=xt[ch][:, 1024 * j : 1024 * (j + 1)],
                                in_=tp.rearrange("p a b -> p (a b)"),
                            )
                        else:
                            nc.vector.tensor_copy(
                                out=xt[ch][:, 1024 * j : 1024 * (j + 1)],
                                in_=tp.rearrange("p a b -> p (a b)"),
                            )

                # ---- softmax + gamma fold ---------------------------------------
                ag = []
                for ch in range(2):
                    mx = sm_pool.tile([P, 1], F32, tag="mx", name=f"mx{ch}_{u}")
                    nc.vector.reduce_max(
                        out=mx, in_=ata[ch][:], axis=mybir.AxisListType.X
                    )
                    negmx = sm_pool.tile([P, 1], F32, tag="negmx", name=f"negmx{ch}_{u}")
                    nc.vector.tensor_scalar_mul(negmx, mx, -1.0)
                    ex = ex_pool.tile([P, C], BF16, tag="ex", name=f"ex{ch}_{u}")
                    ssum = sm_pool.tile([P, 1], F32, tag="ssum", name=f"ssum{ch}_{u}")
                    nc.scalar.activation(
                        out=ex,
                        in_=ata[ch][:],
                        func=mybir.ActivationFunctionType.Exp,
                        bias=negmx,
                        scale=1.0,
                        accum_out=ssum,
                    )
                    rs = sm_pool.tile([P, 1], F32, tag="rs", name=f"rs{ch}_{u}")
                    nc.vector.reciprocal(rs, ssum)
                    grs = sm_pool.tile([P, 1], F32, tag="grs", name=f"grs{ch}_{u}")
                    nc.vector.tensor_mul(grs, rs, gamma_sb)
                    a_g = ex_pool.tile([P, C], BF16, tag="ag", name=f"ag{ch}_{u}")
                    nc.vector.tensor_scalar_mul(a_g, ex, grs)
                    ag.append(a_g)

                # ---- phase 2: out = A @ (gamma*attn) + x ------------------------
                for jj in range(4):
                    outst = outst_pool.tile([P, 8, C], F32, tag="outst", name=f"outst_{u}_{jj}")
                    for kk2 in range(4):
                        k0 = 8 * jj + 2 * kk2
                        op = op_pool.tile(
                            [P, 2, C], F32, tag="op", name=f"op_{u}_{k0}"
                        )
                        for i in range(2):
                            k = k0 + i
                            nc.tensor.matmul(
                                op[:, i, :],
                                lhsT=xt[0][:, k * P : (k + 1) * P],
                                rhs=ag[0],
                                start=True,
                                stop=False,
                            )
                            nc.tensor.matmul(
                                op[:, i, :],
                                lhsT=xt[1][:, k * P : (k + 1) * P],
                                rhs=ag[1],
                                start=False,
                                stop=True,
                            )
                        nc.vector.tensor_add(
                            outst[:, 2 * kk2 : 2 * kk2 + 2, :],
                            op,
                            xf[:, k0 : k0 + 2, :],
                        )
                    nc.sync.dma_start(
                        out=out[b, 1024 * jj : 1024 * (jj + 1), :].rearrange(
                            "(kk p) c -> p kk c", p=P
                        ),
                        in_=outst,
                    )
    nc.compile()
    return nc


_NC_CACHE = {}


def _get_nc(reps: int = 1):
    if reps not in _NC_CACHE:
        _NC_CACHE[reps] = build_bass(reps)
    return _NC_CACHE[reps]


def _shard_inputs(x, gamma):
    x = np.ascontiguousarray(np.asarray(x, dtype=np.float32)).reshape(B_FULL, N, C)
    gamma = np.ascontiguousarray(np.asarray(gamma, dtype=np.float32)).reshape(1)
    in_maps = []
    for i in range(N_CORES):
        in_maps.append(
            {
                "x": np.ascontiguousarray(x[i * B_PER_CORE : (i + 1) * B_PER_CORE]),
                "gamma": gamma,
            }
        )
    return in_maps


def kernel(x, gamma):
    from concourse.bass_utils import run_bass_kernel_spmd

    nc = _get_nc()
    in_maps = _shard_inputs(x, gamma)
    res = run_bass_kernel_spmd(nc, in_maps, core_ids=list(range(N_CORES)))
    out = np.concatenate([r["out"] for r in res.results], axis=0)
    return out.reshape(B_FULL, H, W, C).astype(np.float32, copy=False)
